# revision 2
# baseline (speedup 1.0000x reference)
"""Multi-head causal attention (B=2, S=2048, DIM=2048, H=16, HD=128) with RoPE,
distributed over 8 Trainium2 NeuronCores.

Sharding: data-parallel over batch (2) x tensor-parallel over head groups (4):
core = b*4 + g handles batch b, heads [4g, 4g+4). Each core computes
Q/K/V projections for its head group (bf16 matmuls, fp32 psum), applies RoPE,
runs causal flash-style attention entirely in "d-major" layouts (no on-device
transposes), applies the output projection rows for its heads, and returns a
partial (S, DIM) output. Host sums the 4 partials per batch (row-parallel wo).

Layout tricks:
  - x is fed pre-transposed (xT, dim-major): serves as lhsT for V and as the
    moving operand for Q^T/K^T, so projections directly produce d-major Q^T/K^T.
  - Inputs land in [128, 16*512] "big tiles" (contraction blocks side by side
    in the free dim) via a few 3D-access-pattern DMAs; first x-strip and wq
    stream in eighths on two queues so the first matmul starts ~2us in.
  - RoPE in d-major: rot = raw*C + swap(raw)*S_signed; the pair-swap (d ^ 1)
    is two stride-2-partition SBUF->SBUF DMAs on the gpsimd queue.
  - Scores are computed transposed (S^T tiles, j on partitions), softmax is
    max-free (scores ~ N(0,1): exp never overflows). Phase B processes heads
    in PAIRS: the two heads' score tiles share one 2-bank PSUM tile so a
    single wide 3D-AP exp covers both; the causal mask is a DVE multiply by
    a 0/1 triangle on the diagonal block of the exp'd tile (no mask matmuls).
  - Softmax denominators: ones-column matmuls for the two heads land at
    partitions 0/32 of one PSUM bank -> distinct PE column groups -> the two
    L matmuls execute concurrently (~1 matmul of PE time instead of 2).
  - normalization = scalar copy off PSUM -> gpsimd partition_broadcast ->
    full-width DVE reciprocal -> DVE multiply.
  - The phase-B inner loop is software-pipelined by one j-tile.
  - P^T tiles feed P@V directly; attention output lands d-major (O^T), which
    is exactly the stationary operand the output projection needs.
  - Partial outputs are written bf16 (host sums partials in fp32) as
    [128, 512] chunks DMA'd on alternating queues as soon as each chunk's
    PSUM->SBUF copy lands, so the output drain overlaps phase C compute.
"""

import numpy as np
import ml_dtypes

import concourse.bacc as bacc
import concourse.mybir as mybir
import concourse.tile as tile
from concourse import bass_isa
from concourse.bass_utils import run_bass_kernel_spmd

B, S, DIM, H, HD = 2, 2048, 2048, 16, 128
NCORES = 8
GROUPS = 4               # head groups (tensor-parallel)
HPC = H // GROUPS        # 4 heads per core
GD = HPC * HD            # 512 dims per group
NKT = DIM // 128         # 16 contraction tiles
NSB = S // 512           # 4 s blocks
NIB = S // 512           # 4 i blocks
F32 = mybir.dt.float32
BF16 = mybir.dt.bfloat16
BF = ml_dtypes.bfloat16

_CACHE = {}


def _build():
    nc = bacc.Bacc("TRN2", target_bir_lowering=False, debug=False,
                   num_devices=NCORES)
    xT = nc.dram_tensor("xT", [DIM, S], BF16, kind="ExternalInput").ap()
    wq = nc.dram_tensor("wq", [DIM, GD], BF16, kind="ExternalInput").ap()
    wk = nc.dram_tensor("wk", [DIM, GD], BF16, kind="ExternalInput").ap()
    wv = nc.dram_tensor("wv", [DIM, GD], BF16, kind="ExternalInput").ap()
    wo = nc.dram_tensor("wo", [GD, DIM], BF16, kind="ExternalInput").ap()
    ropeC = nc.dram_tensor("ropeC", [HD, S], BF16, kind="ExternalInput").ap()
    ropeS = nc.dram_tensor("ropeS", [HD, S], BF16, kind="ExternalInput").ap()
    tri01 = nc.dram_tensor("tri01", [128, 128], BF16, kind="ExternalInput").ap()
    out = nc.dram_tensor("out", [S, DIM], BF16, kind="ExternalOutput").ap()

    from contextlib import ExitStack
    with tile.TileContext(nc) as tc:
        with ExitStack() as ctx:
            pool = lambda *a, **k: ctx.enter_context(tc.tile_pool(*a, **k))
            wpool = pool(name="wpool", bufs=1)
            xpool = pool(name="xpool", bufs=2)
            qkpool = pool(name="qkpool", bufs=HPC)
            vpool = pool(name="vpool", bufs=S // 128)
            otpool = pool(name="otpool", bufs=HPC * NIB)
            wopool = pool(name="wopool", bufs=HPC * 4)
            cpool = pool(name="cpool", bufs=1)
            stage = pool(name="stage", bufs=3)
            tpool = pool(name="tpool", bufs=2)
            ptpool = pool(name="ptpool", bufs=3)
            lrpool = pool(name="lrpool", bufs=3)
            bcpool = pool(name="bcpool", bufs=2)
            copool = pool(name="copool", bufs=2)
            ps_mm = pool(name="ps_mm", bufs=3, space="PSUM")
            ps_l = pool(name="ps_l", bufs=1, space="PSUM")
            ps_st = pool(name="ps_st", bufs=2, space="PSUM")
            # ---- weights / constants: emission order = DMA priority.
            # First x-strip (gpsimd queue) + wq (sync queue) in eighths so the
            # first Q matmul can start after ~512KB of DMA; then rope tables,
            # wk, wv; tri01; wo last (phase C only).
            # big-tile layout: all 16 contraction blocks side by side in the
            # free dim ([p, kt*512 + m] = w[kt*128 + p, m]).
            def load_w3d(dst, srcap, eng0, eng1, halves=2):
                nk = NKT // halves
                for hf in range(halves):
                    eng = eng0 if hf % 2 == 0 else eng1
                    eng.dma_start(
                        dst[:, hf * nk * 512:(hf + 1) * nk * 512].rearrange(
                            "p (k m) -> p k m", k=nk),
                        srcap[hf * nk * 128:(hf + 1) * nk * 128, :].rearrange(
                            "(k p) m -> p k m", p=128),
                    )

            wq_all = wpool.tile([128, NKT * 512], BF16, tag="wq")
            xt0 = xpool.tile([128, NKT * 512], BF16, tag="xtb", name="xtb0")
            load_w3d(xt0, xT[:, 0:512], nc.gpsimd, nc.gpsimd, halves=8)
            load_w3d(wq_all, wq, nc.sync, nc.sync, halves=8)
            ropeC_t = cpool.tile([HD, S], BF16, tag="ropeC")
            nc.sync.dma_start(ropeC_t[:], ropeC[:, :])
            ropeS_t = cpool.tile([HD, S], BF16, tag="ropeS")
            nc.sync.dma_start(ropeS_t[:], ropeS[:, :])
            tri01_t = cpool.tile([128, 128], BF16, tag="tri01")
            nc.sync.dma_start(tri01_t[:], tri01[:, :])
            ones_col = cpool.tile([128, 1], BF16, tag="ones_col")
            nc.vector.memset(ones_col[:], 1.0)
            wk_all = wpool.tile([128, NKT * 512], BF16, tag="wk")
            load_w3d(wk_all, wk, nc.sync, nc.sync)
            wv_all = wpool.tile([128, NKT * 512], BF16, tag="wv")
            load_w3d(wv_all, wv, nc.sync, nc.sync)

            # persistent activations (bf16)
            qt_t = [qkpool.tile([128, S], BF16, tag="qt", name=f"qt{h}") for h in range(HPC)]
            kt_t = [qkpool.tile([128, S], BF16, tag="kt", name=f"ktt{h}") for h in range(HPC)]
            v_t = [vpool.tile([128, GD], BF16, tag="v", name=f"v{st}") for st in range(S // 128)]
            ot_t = {}
            for h in range(HPC):
                for ib in range(NIB):
                    ot_t[(h, ib)] = otpool.tile([128, 512], BF16, tag="ot", name=f"ot{h}_{ib}")

            # ---- phase A: projections + rope ----
            for sb in range(NSB):
                s0 = sb * 512
                if sb == 0:
                    xt = xt0
                else:
                    xt = xpool.tile([128, NKT * 512], BF16, tag="xtb",
                                    name=f"xtb{sb}")
                    load_w3d(xt, xT[:, s0:s0 + 512], nc.sync, nc.sync,
                             halves=1)

                for w_all, dst in ((wq_all, qt_t), (wk_all, kt_t)):
                    for h in range(HPC):
                        pmm = ps_mm.tile([128, 512], F32, tag="mm")
                        for kt in range(NKT):
                            k0 = kt * 512
                            nc.tensor.matmul(
                                pmm[:],
                                w_all[:, k0 + h * 128:k0 + (h + 1) * 128],
                                xt[:, k0:k0 + 512],
                                start=(kt == 0), stop=(kt == NKT - 1),
                            )
                        raw = stage.tile([128, 512], BF16, tag="raw")
                        nc.scalar.copy(raw[:], pmm[:])
                        # pair-swap (d ^ 1) via two stride-2-partition
                        # SBUF->SBUF DMAs on the gpsimd queue
                        sw = stage.tile([128, 512], BF16, tag="sw")
                        nc.gpsimd.dma_start(sw[0:128:2, :], raw[1:128:2, :])
                        nc.gpsimd.dma_start(sw[1:128:2, :], raw[0:128:2, :])
                        t1 = tpool.tile([128, 512], BF16, tag="t1")
                        nc.vector.tensor_mul(t1[:], raw[:],
                                             ropeC_t[:, s0:s0 + 512])
                        t2 = tpool.tile([128, 512], BF16, tag="t2")
                        nc.vector.tensor_mul(t2[:], sw[:],
                                             ropeS_t[:, s0:s0 + 512])
                        nc.vector.tensor_add(dst[h][:, s0:s0 + 512],
                                             t1[:], t2[:])

                for st in range(4):
                    pmm = ps_mm.tile([128, 512], F32, tag="mm")
                    for kt in range(NKT):
                        k0 = kt * 512
                        nc.tensor.matmul(
                            pmm[:],
                            xt[:, k0 + st * 128:k0 + (st + 1) * 128],
                            wv_all[:, k0:k0 + 512],
                            start=(kt == 0), stop=(kt == NKT - 1),
                        )
                    nc.vector.tensor_copy(v_t[sb * 4 + st][:], pmm[:])

            # wo loads: needed for phase C; emit now so DMA runs mid-kernel.
            wo_t = {}
            for h in range(HPC):
                for eb in range(4):
                    t = wopool.tile([128, 512], BF16, tag="wo")
                    nc.sync.dma_start(
                        t[:], wo[h * 128:(h + 1) * 128, eb * 512:(eb + 1) * 512]
                    )
                    wo_t[(h, eb)] = t

            # ---- phase B: attention per (i_block, head-pair) ----
            for ib in range(NIB):
                i0 = ib * 512
                njt = 4 * ib + 4
                for hp in range(HPC // 2):
                    h0 = 2 * hp
                    o_ps = [ps_mm.tile([128, 512], F32, tag="mm",
                                       name=f"o{ib}_{hp}_{hx}")
                            for hx in range(2)]
                    l_ps = ps_l.tile([128, 512], F32, tag="l")

                    def emit_lpv(jt, voff, ptp):
                        # L for the two heads at psum partitions 0/32 ->
                        # distinct PE column groups -> concurrent matmuls
                        for hx in range(2):
                            nc.tensor.matmul(
                                l_ps[32 * hx:32 * hx + 1, voff:512],
                                ones_col[:],
                                ptp[:, 512 * hx + voff:512 * hx + 512],
                                start=(jt == 0), stop=(jt == njt - 1),
                                skip_group_check=True,
                            )
                        for hx in range(2):
                            nc.tensor.matmul(
                                o_ps[hx][:, voff:512],
                                v_t[jt][:, (h0 + hx) * 128:(h0 + hx + 1) * 128],
                                ptp[:, 512 * hx + voff:512 * hx + 512],
                                start=(jt == 0), stop=(jt == njt - 1),
                            )

                    # software-pipelined by one jt step: scores/exp for jt
                    # are emitted before L/PV of jt-1, so the tensor queue
                    # always has a scores matmul to run while exp(jt-1)
                    # finishes.
                    prev = None
                    for jt in range(njt):
                        j0 = jt * 128
                        voff = max(0, j0 - i0)
                        st2 = ps_st.tile([128, 1024], F32, tag="st")
                        diag = j0 >= i0
                        for hx in range(2):
                            nc.tensor.matmul(
                                st2[:, 512 * hx + voff:512 * hx + 512],
                                kt_t[h0 + hx][:, j0:j0 + 128],
                                qt_t[h0 + hx][:, i0 + voff:i0 + 512],
                                start=True, stop=True,
                            )
                        ptp = ptpool.tile([128, 1024], BF16, tag="pt")
                        # one wide exp covers both heads via a 3D AP
                        nc.scalar.activation(
                            ptp[:].rearrange("p (two m) -> p two m",
                                             two=2)[:, :, voff:512],
                            st2[:].rearrange("p (two m) -> p two m",
                                             two=2)[:, :, voff:512],
                            mybir.ActivationFunctionType.Exp,
                        )
                        if diag:
                            # causal mask: multiply the diagonal 128x128
                            # block by a 0/1 triangle on the DVE
                            for hx in range(2):
                                nc.vector.tensor_mul(
                                    ptp[:, 512 * hx + voff:
                                        512 * hx + voff + 128],
                                    ptp[:, 512 * hx + voff:
                                        512 * hx + voff + 128],
                                    tri01_t[:],
                                )
                        if prev is not None:
                            emit_lpv(*prev)
                        prev = (jt, voff, ptp)
                    emit_lpv(*prev)

                    # normalization: copy L off PSUM (releases the L bank),
                    # broadcast on gpsimd, then full-width reciprocal on DVE
                    # ([1,512] reciprocal hits a 1-partition slow path).
                    for hx in range(2):
                        lsb = lrpool.tile([1, 512], F32, tag="lrec")
                        nc.scalar.copy(lsb[:], l_ps[32 * hx:32 * hx + 1, :])
                        bc = bcpool.tile([128, 512], F32, tag="bc")
                        nc.gpsimd.partition_broadcast(bc[:], lsb[:],
                                                      channels=128)
                        rec = bcpool.tile([128, 512], F32, tag="rec")
                        nc.vector.reciprocal(rec[:], bc[:])
                        nc.vector.tensor_mul(ot_t[(h0 + hx, ib)][:],
                                             o_ps[hx][:], rec[:])

            # ---- phase C: output projection (partial over this head group) ----
            for stile in range(S // 128):
                ib, soff = stile // 4, (stile % 4) * 128
                co = copool.tile([128, 2048], BF16, tag="co")
                for eb in range(4):
                    pmm = ps_mm.tile([128, 512], F32, tag="mm")
                    for h in range(HPC):
                        nc.tensor.matmul(
                            pmm[:],
                            ot_t[(h, ib)][:, soff:soff + 128],
                            wo_t[(h, eb)][:],
                            start=(h == 0), stop=(h == HPC - 1),
                        )
                    if eb == 0:
                        nc.scalar.copy(co[:, eb * 512:(eb + 1) * 512], pmm[:])
                    else:
                        nc.vector.tensor_copy(
                            co[:, eb * 512:(eb + 1) * 512], pmm[:]
                        )
                    # chunked output DMA on alternating queues: starts the
                    # HBM drain as soon as each 512-col chunk is copied
                    eng = nc.gpsimd if (stile * 4 + eb) % 2 == 0 else nc.sync
                    eng.dma_start(
                        out[stile * 128:(stile + 1) * 128,
                            eb * 512:(eb + 1) * 512],
                        co[:, eb * 512:(eb + 1) * 512],
                    )

    nc.compile()
    return nc


def _host_inputs(x, freqs_cos, freqs_sin, wq, wk, wv, wo):
    """Build the 8 per-core input maps (host-side sharding + layout prep)."""
    scale = 1.0 / np.sqrt(HD)
    # rope tables, d-major duplicated/interleaved: C[d,s]=cos[s,d//2];
    # S[2j,s]=-sin[s,j]; S[2j+1,s]=+sin[s,j]
    c = np.asarray(freqs_cos, dtype=np.float32)      # (S, HD/2)
    s = np.asarray(freqs_sin, dtype=np.float32)
    ropeC = np.repeat(c.T, 2, axis=0)                # (HD, S)
    ropeS = np.empty((HD, S), dtype=np.float32)
    ropeS[0::2] = -s.T
    ropeS[1::2] = s.T
    ropeC = ropeC.astype(BF)
    ropeS = ropeS.astype(BF)

    # 0/1 causal triangle for the diagonal block: keep where j <= i
    tri01 = (np.arange(128)[:, None] <= np.arange(128)[None, :]).astype(
        np.float32).astype(BF)

    xT = [np.ascontiguousarray(np.asarray(x[b]).T).astype(BF) for b in range(B)]
    wq = np.asarray(wq, dtype=np.float32)
    wk = np.asarray(wk, dtype=np.float32)
    wv = np.asarray(wv, dtype=np.float32)
    wo = np.asarray(wo, dtype=np.float32)

    in_maps = []
    for core in range(NCORES):
        b, g = core // GROUPS, core % GROUPS
        cols = slice(g * GD, (g + 1) * GD)
        in_maps.append({
            "xT": xT[b],
            "wq": np.ascontiguousarray(wq[:, cols] * scale).astype(BF),
            "wk": np.ascontiguousarray(wk[:, cols]).astype(BF),
            "wv": np.ascontiguousarray(wv[:, cols]).astype(BF),
            "wo": np.ascontiguousarray(wo[cols, :]).astype(BF),
            "ropeC": ropeC,
            "ropeS": ropeS,
            "tri01": tri01,
        })
    return in_maps


def _get_nc():
    if "nc" not in _CACHE:
        _CACHE["nc"] = _build()
    return _CACHE["nc"]


def run(inputs, trace=False, tmpdir=None):
    """Run on hardware; returns (full_output, BassKernelResults)."""
    nc = _get_nc()
    in_maps = _host_inputs(
        inputs["x"], inputs["freqs_cos"], inputs["freqs_sin"],
        inputs["wq"], inputs["wk"], inputs["wv"], inputs["wo"],
    )
    res = run_bass_kernel_spmd(
        nc, in_maps, core_ids=list(range(NCORES)), trace=trace, tmpdir=tmpdir
    )
    outs = [np.asarray(res.results[c]["out"], dtype=np.float32)
            for c in range(NCORES)]
    full = np.stack(
        [sum(outs[b * GROUPS + g] for g in range(GROUPS)) for b in range(B)],
        axis=0,
    )
    return full, res


def kernel(**inputs) -> np.ndarray:
    full, _ = run(inputs, trace=False)
    return full


# revision 13
# speedup vs baseline: 1.0180x; 1.0180x over previous
"""Multi-head causal attention (B=2, S=2048, DIM=2048, H=16, HD=128) with RoPE,
distributed over 8 Trainium2 NeuronCores.

Sharding: data-parallel over batch (2) x tensor-parallel over head groups (4):
core = b*4 + g handles batch b, heads [4g, 4g+4). Each core computes
Q/K/V projections for its head group (bf16 matmuls, fp32 psum), applies RoPE,
runs causal flash-style attention entirely in "d-major" layouts (no on-device
transposes), applies the output projection rows for its heads, and returns a
partial (S, DIM) output. Host sums the 4 partials per batch (row-parallel wo).

Layout tricks:
  - x is fed pre-transposed (xT, dim-major): serves as lhsT for V and as the
    moving operand for Q^T/K^T, so projections directly produce d-major Q^T/K^T.
  - Inputs land in [128, 16*512] "big tiles" (contraction blocks side by side
    in the free dim) via a few 3D-access-pattern DMAs; first x-strip and wq
    stream in eighths on two queues so the first matmul starts ~2us in.
  - RoPE in d-major: rot = raw*C + swap(raw)*S_signed; the pair-swap (d ^ 1)
    is two stride-2-partition SBUF->SBUF DMAs on the gpsimd queue.
  - Scores are computed transposed (S^T tiles, j on partitions), softmax is
    max-free (scores ~ N(0,1): exp never overflows). Phase B processes heads
    in PAIRS: the two heads' score tiles share one 2-bank PSUM tile so a
    single wide 3D-AP exp covers both; the causal mask is a DVE multiply by
    a 0/1 triangle on the diagonal block of the exp'd tile (no mask matmuls).
  - Softmax denominators: ones-column matmuls for the two heads land at
    partitions 0/32 of one PSUM bank -> distinct PE column groups -> the two
    L matmuls execute concurrently (~1 matmul of PE time instead of 2).
  - normalization = scalar copy off PSUM -> gpsimd partition_broadcast ->
    full-width DVE reciprocal -> DVE multiply.
  - The phase-B inner loop is software-pipelined by one j-tile.
  - P^T tiles feed P@V directly; attention output lands d-major (O^T), which
    is exactly the stationary operand the output projection needs.
  - Partial outputs are written bf16 (host sums partials in fp32) as
    [128, 512] chunks DMA'd on alternating queues as soon as each chunk's
    PSUM->SBUF copy lands, so the output drain overlaps phase C compute.
"""

import numpy as np
import ml_dtypes

import concourse.bacc as bacc
import concourse.mybir as mybir
import concourse.tile as tile
from concourse import bass_isa
from concourse.bass_utils import run_bass_kernel_spmd

B, S, DIM, H, HD = 2, 2048, 2048, 16, 128
NCORES = 8
GROUPS = 4               # head groups (tensor-parallel)
HPC = H // GROUPS        # 4 heads per core
GD = HPC * HD            # 512 dims per group
NKT = DIM // 128         # 16 contraction tiles
NSB = S // 512           # 4 s blocks
NIB = S // 512           # 4 i blocks
F32 = mybir.dt.float32
BF16 = mybir.dt.bfloat16
BF = ml_dtypes.bfloat16

_CACHE = {}


def _build():
    nc = bacc.Bacc("TRN2", target_bir_lowering=False, debug=False,
                   num_devices=NCORES)
    xT = nc.dram_tensor("xT", [DIM, S], BF16, kind="ExternalInput").ap()
    wq = nc.dram_tensor("wq", [DIM, GD], BF16, kind="ExternalInput").ap()
    wk = nc.dram_tensor("wk", [DIM, GD], BF16, kind="ExternalInput").ap()
    wv = nc.dram_tensor("wv", [DIM, GD], BF16, kind="ExternalInput").ap()
    wo = nc.dram_tensor("wo", [GD, DIM], BF16, kind="ExternalInput").ap()
    ropeC = nc.dram_tensor("ropeC", [HD, S], BF16, kind="ExternalInput").ap()
    ropeS = nc.dram_tensor("ropeS", [HD, S], BF16, kind="ExternalInput").ap()
    tri01 = nc.dram_tensor("tri01", [128, 128], BF16, kind="ExternalInput").ap()
    # output stored as [stile, eb, 128, 512] so each 512-col chunk is one
    # fully contiguous 128KB DMA write (full DRAM line efficiency); the host
    # reassembles to (S, DIM).
    out = nc.dram_tensor("out", [S // 128, 4, 128, 512], BF16,
                         kind="ExternalOutput").ap()

    from contextlib import ExitStack
    with tile.TileContext(nc) as tc:
        with ExitStack() as ctx:
            pool = lambda *a, **k: ctx.enter_context(tc.tile_pool(*a, **k))
            wpool = pool(name="wpool", bufs=1)
            xpool = pool(name="xpool", bufs=2)
            qkpool = pool(name="qkpool", bufs=HPC)
            vpool = pool(name="vpool", bufs=S // 128)
            otpool = pool(name="otpool", bufs=HPC * NIB)
            wopool = pool(name="wopool", bufs=HPC * 4)
            cpool = pool(name="cpool", bufs=1)
            stage = pool(name="stage", bufs=2)
            tpool = pool(name="tpool", bufs=2)
            ptpool = pool(name="ptpool", bufs=3)
            lrpool = pool(name="lrpool", bufs=4)
            bcpool = pool(name="bcpool", bufs=1)
            copool = pool(name="copool", bufs=2)
            ps_mm = pool(name="ps_mm", bufs=3, space="PSUM")
            ps_l = pool(name="ps_l", bufs=1, space="PSUM")
            ps_st = pool(name="ps_st", bufs=2, space="PSUM")
            # ---- weights / constants: emission order = DMA priority.
            # First x-strip (gpsimd queue) + wq (sync queue) in eighths so the
            # first Q matmul can start after ~512KB of DMA; then rope tables,
            # wk, wv; tri01; wo last (phase C only).
            # big-tile layout: all 16 contraction blocks side by side in the
            # free dim ([p, kt*512 + m] = w[kt*128 + p, m]).
            def load_w3d(dst, srcap, eng0, eng1, halves=2):
                nk = NKT // halves
                for hf in range(halves):
                    eng = eng0 if hf % 2 == 0 else eng1
                    eng.dma_start(
                        dst[:, hf * nk * 512:(hf + 1) * nk * 512].rearrange(
                            "p (k m) -> p k m", k=nk),
                        srcap[hf * nk * 128:(hf + 1) * nk * 128, :].rearrange(
                            "(k p) m -> p k m", p=128),
                    )

            wq_all = wpool.tile([128, NKT * 512], BF16, tag="wq")
            xt0 = xpool.tile([128, NKT * 512], BF16, tag="xtb", name="xtb0")
            load_w3d(xt0, xT[:, 0:512], nc.gpsimd, nc.gpsimd, halves=8)
            load_w3d(wq_all, wq, nc.sync, nc.sync, halves=8)
            ropeC_t = cpool.tile([HD, S], BF16, tag="ropeC")
            nc.sync.dma_start(ropeC_t[:], ropeC[:, :])
            ropeS_t = cpool.tile([HD, S], BF16, tag="ropeS")
            nc.sync.dma_start(ropeS_t[:], ropeS[:, :])
            tri01_t = cpool.tile([128, 128], BF16, tag="tri01")
            nc.sync.dma_start(tri01_t[:], tri01[:, :])
            ones_col = cpool.tile([128, 1], BF16, tag="ones_col")
            nc.vector.memset(ones_col[:], 1.0)
            wk_all = wpool.tile([128, NKT * 512], BF16, tag="wk")
            load_w3d(wk_all, wk, nc.sync, nc.sync)
            wv_all = wpool.tile([128, NKT * 512], BF16, tag="wv")
            load_w3d(wv_all, wv, nc.sync, nc.sync)

            # persistent activations (bf16)
            qt_t = [qkpool.tile([128, S], BF16, tag="qt", name=f"qt{h}") for h in range(HPC)]
            kt_t = [qkpool.tile([128, S], BF16, tag="kt", name=f"ktt{h}") for h in range(HPC)]
            v_t = [vpool.tile([128, GD], BF16, tag="v", name=f"v{st}") for st in range(S // 128)]
            ot_t = {}
            for h in range(HPC):
                for ib in range(NIB):
                    ot_t[(h, ib)] = otpool.tile([128, 512], BF16, tag="ot", name=f"ot{h}_{ib}")

            # ---- phase A: projections + rope ----
            for sb in range(NSB):
                s0 = sb * 512
                if sb == 0:
                    xt = xt0
                else:
                    xt = xpool.tile([128, NKT * 512], BF16, tag="xtb",
                                    name=f"xtb{sb}")
                    load_w3d(xt, xT[:, s0:s0 + 512], nc.sync, nc.sync,
                             halves=1)

                for w_all, dst in ((wq_all, qt_t), (wk_all, kt_t)):
                    for h in range(HPC):
                        pmm = ps_mm.tile([128, 512], F32, tag="mm")
                        for kt in range(NKT):
                            k0 = kt * 512
                            nc.tensor.matmul(
                                pmm[:],
                                w_all[:, k0 + h * 128:k0 + (h + 1) * 128],
                                xt[:, k0:k0 + 512],
                                start=(kt == 0), stop=(kt == NKT - 1),
                            )
                        raw = stage.tile([128, 512], BF16, tag="raw")
                        nc.scalar.copy(raw[:], pmm[:])
                        # pair-swap (d ^ 1) via two stride-2-partition
                        # SBUF->SBUF DMAs on the gpsimd queue
                        sw = stage.tile([128, 512], BF16, tag="sw")
                        nc.gpsimd.dma_start(sw[0:128:2, :], raw[1:128:2, :])
                        nc.gpsimd.dma_start(sw[1:128:2, :], raw[0:128:2, :])
                        t1 = tpool.tile([128, 512], BF16, tag="t1")
                        nc.vector.tensor_mul(t1[:], raw[:],
                                             ropeC_t[:, s0:s0 + 512])
                        t2 = tpool.tile([128, 512], BF16, tag="t2")
                        nc.vector.tensor_mul(t2[:], sw[:],
                                             ropeS_t[:, s0:s0 + 512])
                        nc.vector.tensor_add(dst[h][:, s0:s0 + 512],
                                             t1[:], t2[:])

                for st in range(4):
                    pmm = ps_mm.tile([128, 512], F32, tag="mm")
                    for kt in range(NKT):
                        k0 = kt * 512
                        nc.tensor.matmul(
                            pmm[:],
                            xt[:, k0 + st * 128:k0 + (st + 1) * 128],
                            wv_all[:, k0:k0 + 512],
                            start=(kt == 0), stop=(kt == NKT - 1),
                        )
                    nc.vector.tensor_copy(v_t[sb * 4 + st][:], pmm[:])

            # wo loads: needed for phase C; emit now so DMA runs mid-kernel.
            wo_t = {}
            for h in range(HPC):
                for eb in range(4):
                    t = wopool.tile([128, 512], BF16, tag="wo")
                    nc.sync.dma_start(
                        t[:], wo[h * 128:(h + 1) * 128, eb * 512:(eb + 1) * 512]
                    )
                    wo_t[(h, eb)] = t

            # ---- phase B: attention per (i_block, head-pair) ----
            # Normalization is decoupled from PSUM release: right after the
            # last PV matmul the unnormalized O is copied off PSUM (frees the
            # bank in <1us), and the broadcast/reciprocal/multiply chain is
            # deferred by one head-pair so DVE backlog never stalls the PE.
            otu_pool = pool(name="otu", bufs=4)
            pending_norm = []

            def flush_norms():
                for (hh, iib, o_unnorm, lsb) in pending_norm:
                    bc = bcpool.tile([128, 512], F32, tag="bc")
                    nc.gpsimd.partition_broadcast(bc[:], lsb[:], channels=128)
                    rec = bcpool.tile([128, 512], F32, tag="rec")
                    nc.vector.reciprocal(rec[:], bc[:])
                    nc.vector.tensor_mul(ot_t[(hh, iib)][:],
                                         o_unnorm[:], rec[:])
                pending_norm.clear()

            for ib in range(NIB):
                i0 = ib * 512
                njt = 4 * ib + 4
                for hp in range(HPC // 2):
                    h0 = 2 * hp
                    o_ps = [ps_mm.tile([128, 512], F32, tag="mm",
                                       name=f"o{ib}_{hp}_{hx}")
                            for hx in range(2)]
                    l_ps = ps_l.tile([128, 512], F32, tag="l")

                    def emit_lpv(jt, voff, ptp):
                        # L for the two heads at psum partitions 0/32 ->
                        # distinct PE column groups -> concurrent matmuls
                        for hx in range(2):
                            nc.tensor.matmul(
                                l_ps[32 * hx:32 * hx + 1, voff:512],
                                ones_col[:],
                                ptp[:, 512 * hx + voff:512 * hx + 512],
                                start=(jt == 0), stop=(jt == njt - 1),
                                skip_group_check=True,
                            )
                        for hx in range(2):
                            nc.tensor.matmul(
                                o_ps[hx][:, voff:512],
                                v_t[jt][:, (h0 + hx) * 128:(h0 + hx + 1) * 128],
                                ptp[:, 512 * hx + voff:512 * hx + 512],
                                start=(jt == 0), stop=(jt == njt - 1),
                            )

                    # software-pipelined by one jt step: scores/exp for jt
                    # are emitted before L/PV of jt-1, so the tensor queue
                    # always has a scores matmul to run while exp(jt-1)
                    # finishes.
                    prev = None
                    for jt in range(njt):
                        j0 = jt * 128
                        voff = max(0, j0 - i0)
                        st2 = ps_st.tile([128, 1024], F32, tag="st")
                        diag = j0 >= i0
                        for hx in range(2):
                            nc.tensor.matmul(
                                st2[:, 512 * hx + voff:512 * hx + 512],
                                kt_t[h0 + hx][:, j0:j0 + 128],
                                qt_t[h0 + hx][:, i0 + voff:i0 + 512],
                                start=True, stop=True,
                            )
                        ptp = ptpool.tile([128, 1024], BF16, tag="pt")
                        # one wide exp covers both heads via a 3D AP
                        nc.scalar.activation(
                            ptp[:].rearrange("p (two m) -> p two m",
                                             two=2)[:, :, voff:512],
                            st2[:].rearrange("p (two m) -> p two m",
                                             two=2)[:, :, voff:512],
                            mybir.ActivationFunctionType.Exp,
                        )
                        if diag:
                            # causal mask: multiply the diagonal 128x128
                            # block by a 0/1 triangle on the DVE
                            for hx in range(2):
                                nc.vector.tensor_mul(
                                    ptp[:, 512 * hx + voff:
                                        512 * hx + voff + 128],
                                    ptp[:, 512 * hx + voff:
                                        512 * hx + voff + 128],
                                    tri01_t[:],
                                )
                        if prev is not None:
                            emit_lpv(*prev)
                        prev = (jt, voff, ptp)
                    emit_lpv(*prev)

                    # copy L and unnormalized O off PSUM now (releases both
                    # banks quickly); the reciprocal chain runs a pair later.
                    for hx in range(2):
                        lsb = lrpool.tile([1, 512], F32, tag="lrec")
                        nc.scalar.copy(lsb[:], l_ps[32 * hx:32 * hx + 1, :])
                        otu = otu_pool.tile([128, 512], BF16, tag="otu")
                        nc.scalar.copy(otu[:], o_ps[hx][:])
                        pending_norm.append((h0 + hx, ib, otu, lsb))
                    if len(pending_norm) >= 4:
                        flush_norms()
            flush_norms()

            # ---- phase C: output projection (partial over this head group) ----
            for stile in range(S // 128):
                ib, soff = stile // 4, (stile % 4) * 128
                co = copool.tile([128, 2048], BF16, tag="co")
                for eb in range(4):
                    pmm = ps_mm.tile([128, 512], F32, tag="mm")
                    for h in range(HPC):
                        nc.tensor.matmul(
                            pmm[:],
                            ot_t[(h, ib)][:, soff:soff + 128],
                            wo_t[(h, eb)][:],
                            start=(h == 0), stop=(h == HPC - 1),
                        )
                    if eb == 0:
                        nc.scalar.copy(co[:, eb * 512:(eb + 1) * 512], pmm[:])
                    else:
                        nc.vector.tensor_copy(
                            co[:, eb * 512:(eb + 1) * 512], pmm[:]
                        )
                    # chunked output DMA on alternating queues: each chunk is
                    # a fully contiguous 128KB write, issued as soon as its
                    # PSUM->SBUF copy lands
                    eng = nc.gpsimd if (stile * 4 + eb) % 2 == 0 else nc.sync
                    eng.dma_start(
                        out[stile, eb],
                        co[:, eb * 512:(eb + 1) * 512],
                    )

    nc.compile()
    return nc


def _host_inputs(x, freqs_cos, freqs_sin, wq, wk, wv, wo):
    """Build the 8 per-core input maps (host-side sharding + layout prep)."""
    scale = 1.0 / np.sqrt(HD)
    # rope tables, d-major duplicated/interleaved: C[d,s]=cos[s,d//2];
    # S[2j,s]=-sin[s,j]; S[2j+1,s]=+sin[s,j]
    c = np.asarray(freqs_cos, dtype=np.float32)      # (S, HD/2)
    s = np.asarray(freqs_sin, dtype=np.float32)
    ropeC = np.repeat(c.T, 2, axis=0)                # (HD, S)
    ropeS = np.empty((HD, S), dtype=np.float32)
    ropeS[0::2] = -s.T
    ropeS[1::2] = s.T
    ropeC = ropeC.astype(BF)
    ropeS = ropeS.astype(BF)

    # 0/1 causal triangle for the diagonal block: keep where j <= i
    tri01 = (np.arange(128)[:, None] <= np.arange(128)[None, :]).astype(
        np.float32).astype(BF)

    xT = [np.ascontiguousarray(np.asarray(x[b]).T).astype(BF) for b in range(B)]
    wq = np.asarray(wq, dtype=np.float32)
    wk = np.asarray(wk, dtype=np.float32)
    wv = np.asarray(wv, dtype=np.float32)
    wo = np.asarray(wo, dtype=np.float32)

    in_maps = []
    for core in range(NCORES):
        b, g = core // GROUPS, core % GROUPS
        cols = slice(g * GD, (g + 1) * GD)
        in_maps.append({
            "xT": xT[b],
            "wq": np.ascontiguousarray(wq[:, cols] * scale).astype(BF),
            "wk": np.ascontiguousarray(wk[:, cols]).astype(BF),
            "wv": np.ascontiguousarray(wv[:, cols]).astype(BF),
            "wo": np.ascontiguousarray(wo[cols, :]).astype(BF),
            "ropeC": ropeC,
            "ropeS": ropeS,
            "tri01": tri01,
        })
    return in_maps


def _get_nc():
    if "nc" not in _CACHE:
        _CACHE["nc"] = _build()
    return _CACHE["nc"]


def run(inputs, trace=False, tmpdir=None):
    """Run on hardware; returns (full_output, BassKernelResults)."""
    nc = _get_nc()
    in_maps = _host_inputs(
        inputs["x"], inputs["freqs_cos"], inputs["freqs_sin"],
        inputs["wq"], inputs["wk"], inputs["wv"], inputs["wo"],
    )
    res = run_bass_kernel_spmd(
        nc, in_maps, core_ids=list(range(NCORES)), trace=trace, tmpdir=tmpdir
    )
    # device layout [stile, eb, 128, 512] -> (S, DIM)
    outs = [np.asarray(res.results[c]["out"], dtype=np.float32)
            .transpose(0, 2, 1, 3).reshape(S, DIM)
            for c in range(NCORES)]
    full = np.stack(
        [sum(outs[b * GROUPS + g] for g in range(GROUPS)) for b in range(B)],
        axis=0,
    )
    return full, res


def kernel(**inputs) -> np.ndarray:
    full, _ = run(inputs, trace=False)
    return full


# revision 20
# speedup vs baseline: 1.1221x; 1.1022x over previous
"""Multi-head causal attention (B=2, S=2048, DIM=2048, H=16, HD=128) with RoPE,
distributed over 8 Trainium2 NeuronCores.

Sharding: data-parallel over batch (2) x tensor-parallel over head groups (4):
core = b*4 + g handles batch b, heads [4g, 4g+4). Each core computes
Q/K/V projections for its head group (bf16 matmuls, fp32 psum), applies RoPE,
runs causal flash-style attention entirely in "d-major" layouts (no on-device
transposes), applies the output projection rows for its heads, and returns a
partial (S, DIM) output. Host sums the 4 partials per batch (row-parallel wo).

Layout tricks:
  - x is fed pre-transposed (xT, dim-major): serves as lhsT for V and as the
    moving operand for Q^T/K^T, so projections directly produce d-major Q^T/K^T.
  - Inputs land in [128, 16*512] "big tiles" (contraction blocks side by side
    in the free dim) via a few 3D-access-pattern DMAs; first x-strip and wq
    stream in eighths on two queues so the first matmul starts ~2us in.
  - RoPE in d-major: rot = raw*C + swap(raw)*S_signed; the pair-swap (d ^ 1)
    is two stride-2-partition SBUF->SBUF DMAs on the gpsimd queue.
  - Scores are computed transposed (S^T tiles, j on partitions), softmax is
    max-free (scores ~ N(0,1): exp never overflows). Phase B processes heads
    in PAIRS: the two heads' score tiles share one 2-bank PSUM tile so a
    single wide 3D-AP exp covers both; the causal mask is a DVE multiply by
    a 0/1 triangle on the diagonal block of the exp'd tile (no mask matmuls).
  - Softmax denominators: ones-column matmuls for the two heads land at
    partitions 0/32 of one PSUM bank -> distinct PE column groups -> the two
    L matmuls execute concurrently (~1 matmul of PE time instead of 2).
  - normalization = scalar copy off PSUM -> gpsimd partition_broadcast ->
    full-width DVE reciprocal -> DVE multiply.
  - The phase-B inner loop is software-pipelined by one j-tile.
  - P^T tiles feed P@V directly; attention output lands d-major (O^T), which
    is exactly the stationary operand the output projection needs.
  - Partial outputs are written bf16 (host sums partials in fp32) as
    [128, 512] chunks DMA'd on alternating queues as soon as each chunk's
    PSUM->SBUF copy lands, so the output drain overlaps phase C compute.
"""

import numpy as np
import ml_dtypes

import concourse.bacc as bacc
import concourse.mybir as mybir
import concourse.tile as tile
from concourse import bass_isa
from concourse.bass_utils import run_bass_kernel_spmd

B, S, DIM, H, HD = 2, 2048, 2048, 16, 128
NCORES = 8
GROUPS = 4               # head groups (tensor-parallel)
HPC = H // GROUPS        # 4 heads per core
GD = HPC * HD            # 512 dims per group
NKT = DIM // 128         # 16 contraction tiles
NSB = S // 512           # 4 s blocks
NIB = S // 512           # 4 i blocks
F32 = mybir.dt.float32
BF16 = mybir.dt.bfloat16
BF = ml_dtypes.bfloat16

_CACHE = {}


def _build():
    nc = bacc.Bacc("TRN2", target_bir_lowering=False, debug=False,
                   num_devices=NCORES)
    xT = nc.dram_tensor("xT", [DIM, S], BF16, kind="ExternalInput").ap()
    wq = nc.dram_tensor("wq", [DIM, GD], BF16, kind="ExternalInput").ap()
    wk = nc.dram_tensor("wk", [DIM, GD], BF16, kind="ExternalInput").ap()
    wv = nc.dram_tensor("wv", [DIM, GD], BF16, kind="ExternalInput").ap()
    wo = nc.dram_tensor("wo", [GD, DIM], BF16, kind="ExternalInput").ap()
    ropeC = nc.dram_tensor("ropeC", [HD, S], BF16, kind="ExternalInput").ap()
    ropeS = nc.dram_tensor("ropeS", [HD, S], BF16, kind="ExternalInput").ap()
    tri01 = nc.dram_tensor("tri01", [128, 128], BF16, kind="ExternalInput").ap()
    # output stored as [stile, eb, 128, 512] so each 512-col chunk is one
    # fully contiguous 128KB DMA write (full DRAM line efficiency); the host
    # reassembles to (S, DIM).
    out = nc.dram_tensor("out", [S // 128, 4, 128, 512], BF16,
                         kind="ExternalOutput").ap()

    from contextlib import ExitStack
    with tile.TileContext(nc) as tc:
        with ExitStack() as ctx:
            pool = lambda *a, **k: ctx.enter_context(tc.tile_pool(*a, **k))
            wpool = pool(name="wpool", bufs=1)
            xpool = pool(name="xpool", bufs=2)
            qkpool = pool(name="qkpool", bufs=HPC)
            vpool = pool(name="vpool", bufs=S // 128)
            otpool = pool(name="otpool", bufs=HPC * NIB)
            wopool = pool(name="wopool", bufs=HPC * 4)
            cpool = pool(name="cpool", bufs=1)
            stage = pool(name="stage", bufs=3)
            tpool = pool(name="tpool", bufs=2)
            ptpool = pool(name="ptpool", bufs=4)
            lrpool = pool(name="lrpool", bufs=3)
            bcpool = pool(name="bcpool", bufs=1)
            copool = pool(name="copool", bufs=6)
            ps_mm = pool(name="ps_mm", bufs=3, space="PSUM")
            ps_l = pool(name="ps_l", bufs=1, space="PSUM")
            ps_st = pool(name="ps_st", bufs=2, space="PSUM")
            # ---- weights / constants: emission order = DMA priority.
            # First x-strip (gpsimd queue) + wq (sync queue) in eighths so the
            # first Q matmul can start after ~512KB of DMA; then rope tables,
            # wk, wv; tri01; wo last (phase C only).
            # big-tile layout: all 16 contraction blocks side by side in the
            # free dim ([p, kt*512 + m] = w[kt*128 + p, m]).
            def load_w3d(dst, srcap, eng0, eng1, halves=2):
                nk = NKT // halves
                for hf in range(halves):
                    eng = eng0 if hf % 2 == 0 else eng1
                    eng.dma_start(
                        dst[:, hf * nk * 512:(hf + 1) * nk * 512].rearrange(
                            "p (k m) -> p k m", k=nk),
                        srcap[hf * nk * 128:(hf + 1) * nk * 128, :].rearrange(
                            "(k p) m -> p k m", p=128),
                    )

            wq_all = wpool.tile([128, NKT * 512], BF16, tag="wq")
            xt0 = xpool.tile([128, NKT * 512], BF16, tag="xtb", name="xtb0")
            load_w3d(xt0, xT[:, 0:512], nc.gpsimd, nc.gpsimd, halves=8)
            load_w3d(wq_all, wq, nc.sync, nc.sync, halves=8)
            ropeC_t = cpool.tile([HD, S], BF16, tag="ropeC")
            nc.sync.dma_start(ropeC_t[:], ropeC[:, :])
            ropeS_t = cpool.tile([HD, S], BF16, tag="ropeS")
            nc.sync.dma_start(ropeS_t[:], ropeS[:, :])
            tri01_t = cpool.tile([128, 128], BF16, tag="tri01")
            nc.sync.dma_start(tri01_t[:], tri01[:, :])
            ones_col = cpool.tile([128, 1], BF16, tag="ones_col")
            nc.vector.memset(ones_col[:], 1.0)
            wk_all = wpool.tile([128, NKT * 512], BF16, tag="wk")
            load_w3d(wk_all, wk, nc.sync, nc.sync)
            wv_all = wpool.tile([128, NKT * 512], BF16, tag="wv")
            load_w3d(wv_all, wv, nc.sync, nc.sync)

            # persistent activations (bf16)
            qt_t = [qkpool.tile([128, S], BF16, tag="qt", name=f"qt{h}") for h in range(HPC)]
            kt_t = [qkpool.tile([128, S], BF16, tag="kt", name=f"ktt{h}") for h in range(HPC)]
            v_t = [vpool.tile([128, GD], BF16, tag="v", name=f"v{st}") for st in range(S // 128)]
            ot_t = {}
            for h in range(HPC):
                for ib in range(NIB):
                    ot_t[(h, ib)] = otpool.tile([128, 512], BF16, tag="ot", name=f"ot{h}_{ib}")

            # ---- phase A: projections + rope ----
            for sb in range(NSB):
                s0 = sb * 512
                if sb == 0:
                    xt = xt0
                else:
                    xt = xpool.tile([128, NKT * 512], BF16, tag="xtb",
                                    name=f"xtb{sb}")
                    load_w3d(xt, xT[:, s0:s0 + 512], nc.sync, nc.sync,
                             halves=1)

                for w_all, dst in ((wq_all, qt_t), (wk_all, kt_t)):
                    for h in range(HPC):
                        pmm = ps_mm.tile([128, 512], F32, tag="mm")
                        for kt in range(NKT):
                            k0 = kt * 512
                            nc.tensor.matmul(
                                pmm[:],
                                w_all[:, k0 + h * 128:k0 + (h + 1) * 128],
                                xt[:, k0:k0 + 512],
                                start=(kt == 0), stop=(kt == NKT - 1),
                            )
                        raw = stage.tile([128, 512], BF16, tag="raw")
                        nc.scalar.copy(raw[:], pmm[:])
                        # pair-swap (d ^ 1) via two stride-2-partition
                        # SBUF->SBUF DMAs on the gpsimd queue
                        sw = stage.tile([128, 512], BF16, tag="sw")
                        nc.gpsimd.dma_start(sw[0:128:2, :], raw[1:128:2, :])
                        nc.gpsimd.dma_start(sw[1:128:2, :], raw[0:128:2, :])
                        t1 = tpool.tile([128, 512], BF16, tag="t1")
                        nc.vector.tensor_mul(t1[:], raw[:],
                                             ropeC_t[:, s0:s0 + 512])
                        t2 = tpool.tile([128, 512], BF16, tag="t2")
                        nc.vector.tensor_mul(t2[:], sw[:],
                                             ropeS_t[:, s0:s0 + 512])
                        nc.vector.tensor_add(dst[h][:, s0:s0 + 512],
                                             t1[:], t2[:])

                for st in range(4):
                    pmm = ps_mm.tile([128, 512], F32, tag="mm")
                    for kt in range(NKT):
                        k0 = kt * 512
                        nc.tensor.matmul(
                            pmm[:],
                            xt[:, k0 + st * 128:k0 + (st + 1) * 128],
                            wv_all[:, k0:k0 + 512],
                            start=(kt == 0), stop=(kt == NKT - 1),
                        )
                    nc.vector.tensor_copy(v_t[sb * 4 + st][:], pmm[:])

            # wo loads: needed for phase C; emit now so DMA runs mid-kernel.
            wo_t = {}
            for h in range(HPC):
                for eb in range(4):
                    t = wopool.tile([128, 512], BF16, tag="wo")
                    nc.sync.dma_start(
                        t[:], wo[h * 128:(h + 1) * 128, eb * 512:(eb + 1) * 512]
                    )
                    wo_t[(h, eb)] = t

            # ---- phase B: attention over a flat (i_block, head-pair, jt)
            # stream with a depth-3 software pipeline that crosses unit
            # boundaries: scores/exp for the next unit are emitted before the
            # last L/PV + PSUM-release copies of the previous one, so the
            # scalar queue always has the next exp in front and the PE never
            # drains at a unit boundary. Normalization (broadcast +
            # fast-approx reciprocal + multiply) is deferred one unit.
            from collections import deque
            otu_pool = pool(name="otu", bufs=3)
            pending_norm = []

            def flush_norms():
                for (hh, iib, o_unnorm, lsb) in pending_norm:
                    bc = bcpool.tile([128, 512], F32, tag="bc")
                    nc.gpsimd.partition_broadcast(bc[:], lsb[:], channels=128)
                    rec = bcpool.tile([128, 512], F32, tag="rec")
                    nc.vector.reciprocal_approx_fast(rec[:], bc[:])
                    nc.vector.tensor_mul(ot_t[(hh, iib)][:],
                                         o_unnorm[:], rec[:])
                pending_norm.clear()

            def emit_scores_exp(us, jt):
                i0, h0 = us["i0"], us["h0"]
                j0 = jt * 128
                voff = max(0, j0 - i0)
                st2 = ps_st.tile([128, 1024], F32, tag="st")
                for hx in range(2):
                    nc.tensor.matmul(
                        st2[:, 512 * hx + voff:512 * hx + 512],
                        kt_t[h0 + hx][:, j0:j0 + 128],
                        qt_t[h0 + hx][:, i0 + voff:i0 + 512],
                        start=True, stop=True,
                    )
                ptp = ptpool.tile([128, 1024], BF16, tag="pt")
                # one wide exp covers both heads via a 3D AP
                nc.scalar.activation(
                    ptp[:].rearrange("p (two m) -> p two m",
                                     two=2)[:, :, voff:512],
                    st2[:].rearrange("p (two m) -> p two m",
                                     two=2)[:, :, voff:512],
                    mybir.ActivationFunctionType.Exp,
                )
                if j0 >= i0:
                    # causal mask: multiply the diagonal 128x128 block by a
                    # 0/1 triangle on the DVE
                    for hx in range(2):
                        nc.vector.tensor_mul(
                            ptp[:, 512 * hx + voff:512 * hx + voff + 128],
                            ptp[:, 512 * hx + voff:512 * hx + voff + 128],
                            tri01_t[:],
                        )
                return (us, jt, voff, ptp)

            def emit_lpv(us, jt, voff, ptp):
                njt, h0 = us["njt"], us["h0"]
                # L for the two heads at psum partitions 0/32 -> distinct PE
                # column groups -> the two matmuls execute concurrently
                for hx in range(2):
                    nc.tensor.matmul(
                        us["l_ps"][32 * hx:32 * hx + 1, voff:512],
                        ones_col[:],
                        ptp[:, 512 * hx + voff:512 * hx + 512],
                        start=(jt == 0), stop=(jt == njt - 1),
                        skip_group_check=True,
                    )
                for hx in range(2):
                    nc.tensor.matmul(
                        us["o_ps"][hx][:, voff:512],
                        v_t[jt][:, (h0 + hx) * 128:(h0 + hx + 1) * 128],
                        ptp[:, 512 * hx + voff:512 * hx + 512],
                        start=(jt == 0), stop=(jt == njt - 1),
                    )
                if jt == njt - 1:
                    # unit retires: emit the older units' deferred norm
                    # chains, then copy L and unnormalized O off PSUM
                    # (releases the banks; runs behind the next unit's
                    # already-queued exps on the scalar engine)
                    flush_norms()
                    for hx in range(2):
                        lsb = lrpool.tile([1, 512], F32, tag="lrec")
                        nc.scalar.copy(lsb[:],
                                       us["l_ps"][32 * hx:32 * hx + 1, :])
                        otu = otu_pool.tile([128, 512], BF16, tag="otu")
                        nc.scalar.copy(otu[:], us["o_ps"][hx][:])
                        pending_norm.append((us["h0"] + hx, us["ib"],
                                             otu, lsb))

            pipe = deque()
            for ib in range(NIB):
                for hp in range(HPC // 2):
                    us = {
                        "ib": ib, "i0": ib * 512, "h0": 2 * hp,
                        "njt": 4 * ib + 4,
                        "o_ps": [ps_mm.tile([128, 512], F32, tag="mm",
                                            name=f"o{ib}_{hp}_{hx}")
                                 for hx in range(2)],
                        "l_ps": ps_l.tile([128, 512], F32, tag="l",
                                          name=f"l{ib}_{hp}"),
                    }
                    for jt in range(us["njt"]):
                        pipe.append(emit_scores_exp(us, jt))
                        if len(pipe) > 2:
                            emit_lpv(*pipe.popleft())
            while pipe:
                emit_lpv(*pipe.popleft())
            flush_norms()

            # ---- phase C: output projection (partial over this head group) ----
            for stile in range(S // 128):
                ib, soff = stile // 4, (stile % 4) * 128
                for eb in range(4):
                    pmm = ps_mm.tile([128, 512], F32, tag="mm")
                    for h in range(HPC):
                        nc.tensor.matmul(
                            pmm[:],
                            ot_t[(h, ib)][:, soff:soff + 128],
                            wo_t[(h, eb)][:],
                            start=(h == 0), stop=(h == HPC - 1),
                        )
                    co = copool.tile([128, 512], BF16, tag="co")
                    if eb == 0:
                        nc.scalar.copy(co[:], pmm[:])
                    else:
                        nc.vector.tensor_copy(co[:], pmm[:])
                    # chunked output DMA on alternating queues: each chunk is
                    # a fully contiguous 128KB write, issued as soon as its
                    # PSUM->SBUF copy lands
                    eng = nc.gpsimd if (stile * 4 + eb) % 2 == 0 else nc.sync
                    eng.dma_start(out[stile, eb], co[:])

    nc.compile()
    return nc


def _host_inputs(x, freqs_cos, freqs_sin, wq, wk, wv, wo):
    """Build the 8 per-core input maps (host-side sharding + layout prep)."""
    scale = 1.0 / np.sqrt(HD)
    # rope tables, d-major duplicated/interleaved: C[d,s]=cos[s,d//2];
    # S[2j,s]=-sin[s,j]; S[2j+1,s]=+sin[s,j]
    c = np.asarray(freqs_cos, dtype=np.float32)      # (S, HD/2)
    s = np.asarray(freqs_sin, dtype=np.float32)
    ropeC = np.repeat(c.T, 2, axis=0)                # (HD, S)
    ropeS = np.empty((HD, S), dtype=np.float32)
    ropeS[0::2] = -s.T
    ropeS[1::2] = s.T
    ropeC = ropeC.astype(BF)
    ropeS = ropeS.astype(BF)

    # 0/1 causal triangle for the diagonal block: keep where j <= i
    tri01 = (np.arange(128)[:, None] <= np.arange(128)[None, :]).astype(
        np.float32).astype(BF)

    xT = [np.ascontiguousarray(np.asarray(x[b]).T).astype(BF) for b in range(B)]
    wq = np.asarray(wq, dtype=np.float32)
    wk = np.asarray(wk, dtype=np.float32)
    wv = np.asarray(wv, dtype=np.float32)
    wo = np.asarray(wo, dtype=np.float32)

    in_maps = []
    for core in range(NCORES):
        b, g = core // GROUPS, core % GROUPS
        cols = slice(g * GD, (g + 1) * GD)
        in_maps.append({
            "xT": xT[b],
            "wq": np.ascontiguousarray(wq[:, cols] * scale).astype(BF),
            "wk": np.ascontiguousarray(wk[:, cols]).astype(BF),
            "wv": np.ascontiguousarray(wv[:, cols]).astype(BF),
            "wo": np.ascontiguousarray(wo[cols, :]).astype(BF),
            "ropeC": ropeC,
            "ropeS": ropeS,
            "tri01": tri01,
        })
    return in_maps


def _get_nc():
    if "nc" not in _CACHE:
        _CACHE["nc"] = _build()
    return _CACHE["nc"]


def run(inputs, trace=False, tmpdir=None):
    """Run on hardware; returns (full_output, BassKernelResults)."""
    nc = _get_nc()
    in_maps = _host_inputs(
        inputs["x"], inputs["freqs_cos"], inputs["freqs_sin"],
        inputs["wq"], inputs["wk"], inputs["wv"], inputs["wo"],
    )
    res = run_bass_kernel_spmd(
        nc, in_maps, core_ids=list(range(NCORES)), trace=trace, tmpdir=tmpdir
    )
    # device layout [stile, eb, 128, 512] -> (S, DIM)
    outs = [np.asarray(res.results[c]["out"], dtype=np.float32)
            .transpose(0, 2, 1, 3).reshape(S, DIM)
            for c in range(NCORES)]
    full = np.stack(
        [sum(outs[b * GROUPS + g] for g in range(GROUPS)) for b in range(B)],
        axis=0,
    )
    return full, res


def kernel(**inputs) -> np.ndarray:
    full, _ = run(inputs, trace=False)
    return full
